# revision 38
# baseline (speedup 1.0000x reference)
"""Manual-sync (raw bacc) Trainium2 kernel for fused cosine-distance row merge.

Per row i: out[i] = u_i*A[i] + (1-u_i)*B[i], u_i = 0.5 - 0.5*dot_i/scale_i,
scale_i = max(|A_i||B_i|, 1e-8).

bf16 I/O: host converts f32 inputs to bf16 (rel-err ~2e-3, well under the
2e-2 gate), halving HBM traffic: 8MB loads + 4MB stores per core.

Custom DVE ops with hand-authored 2X_1PORT uop programs (the stock lower()
only emits the 1x program; hardware-measured per [128,1024] bf16 subtile):
  - PRODSCAN2X: prefix-scan of Src0*Src1. The out AP is a [P,2] slot
    broadcast to [P,512,2] (outer stride 0), so the final written word is
    the full row dot/sum-of-squares in bf16. ~692ns vs 1305ns for the 1x
    scalar_tensor_tensor+accum path. Used for dots and for ssB on part of
    the units.
  - LERP2X: (in0-in1)*(imm2-s0*s1)+in1 at 2x: ~811ns vs 1344ns.
Both ops also carry the stock-lowered 1x program, so a silent perf-mode
fallback still computes correct values.

Engine assignment (balanced ~33us each, at the ~33.5us/core DMA floor):
  - DVE: all 16 dots, ssB for units 0,1,4,5 (10 subtiles), all 16 lerps,
    per-unit stats (dot->f32 copy, sc=ssA*ssB, clamp, reciprocal).
  - ACT: ssA for all units + ssB for units 2,3 (square activation with
    f32 accum), per-unit sqrt.
  - DVE runs one unit ahead on dots/ssB; stats+lerps of unit i-1 are
    emitted after the dots of unit i so DVE never stalls on ACT.
  - Loads: HWDGE (SP) back-to-back from t=0. Stores: SWDGE (gpsimd) after
    all loads (avoids chip-wide HBM read/write mixing), per-unit once that
    unit's lerps retire.
"""

import copy

import ml_dtypes
import numpy as np

import concourse.bacc as bacc
import concourse.mybir as mybir
from concourse import dve_ops
from concourse.dve_spec import (
    Spec, Src0, Src1, C0, C1, C2, lower, _has_src1, scan, AluOp as SAluOp,
)
from concourse.dve_uop import (
    DveOpSpec, UopConfig, InpSel, OutSel, OutPath, AluInp, DelayInp,
    Trigger, AluOp, ENABLE,
)

N_FULL = 16384
D = 1024
NCORES = 8
ROWS = N_FULL // NCORES  # 2048
P = 128
EPS = 1e-8

F32 = mybir.dt.float32
BF16 = mybir.dt.bfloat16
NP_BF16 = ml_dtypes.bfloat16


# ---------------------------------------------------------------------------
# Custom DVE ops with 2X_1PORT programs
# ---------------------------------------------------------------------------

def _lerp2x_steady():
    """2x steady: lo result rides delay chain 0 to the end, hi at ALU out."""
    u = UopConfig()
    u.enable_input(InpSel.SRC_0, 1)     # -> d0
    u.enable_input(InpSel.SRC_1, 2)     # -> d1
    u.enable_input(InpSel.CONST_2, 3)   # -> d2
    u.enable_input(InpSel.SRC_0_HI, 4)  # -> d3
    u.enable_input(InpSel.SRC_1_HI, 5)  # -> d4
    dp = u.datapath_config
    dp[0].enable_alu(AluOp.SUBTRACT, AluInp.PREV_DELAY_0, AluInp.PREV_DELAY_1)
    dp[0].pass_through_delay(1, 2, 3, 4)
    # w = imm2 - s0*s1 (s0*s1 latched into this slice's swap flop by uop0)
    dp[1].enable_alu(AluOp.SUBTRACT, AluInp.PREV_DELAY_2, AluInp.CURR_SWAP_OUT)
    dp[1].enable_delay_from_src(DelayInp.PREV_ALU_OUT, 0)
    dp[1].pass_through_delay(1, 3, 4)
    dp[2].enable_alu(AluOp.MULTIPLY, AluInp.PREV_DELAY_0, AluInp.PREV_ALU_OUT)
    dp[2].enable_delay_from_src(DelayInp.PREV_ALU_OUT, 2)
    dp[2].pass_through_delay(1, 3, 4)
    dp[3].enable_alu(AluOp.ADD, AluInp.PREV_ALU_OUT, AluInp.PREV_DELAY_1)
    dp[3].pass_through_delay(2, 3, 4)
    dp[4].enable_alu(AluOp.SUBTRACT, AluInp.PREV_DELAY_3, AluInp.PREV_DELAY_4)
    dp[4].enable_delay_from_src(DelayInp.PREV_ALU_OUT, 0)
    dp[4].pass_through_delay(2, 4)
    dp[5].enable_alu(AluOp.MULTIPLY, AluInp.PREV_ALU_OUT, AluInp.PREV_DELAY_2)
    dp[5].pass_through_delay(0, 4)
    dp[6].enable_alu(AluOp.ADD, AluInp.PREV_ALU_OUT, AluInp.PREV_DELAY_4)
    dp[6].pass_through_delay(0)
    dp[7].pass_through_alu()
    dp[7].pass_through_delay(0)
    u.enable_output(OutSel.DELAY_0, OutPath.WR0_LO)
    u.enable_output(OutSel.ALU_OUT, OutPath.WR0_HI)
    u.require_inp0 = ENABLE
    u.require_inp1 = ENABLE
    u.trigger = (Trigger.SRC_TENSOR_DONE, Trigger.NONE, Trigger.NONE)
    u.next_uop = (0, 0, 0)
    return u


def _prodscan2x_seed():
    """Runs once before the stream (consumes nothing): zero the feedback
    flops at blk1 (lo partial sum) and blk3 (hi partial sum)."""
    u = UopConfig()
    u.enable_input(InpSel.ZERO, 3)  # -> d2
    dp = u.datapath_config
    dp[0].pass_through_delay(2)
    dp[1].enable_alu(AluOp.BYPASS, AluInp.PREV_DELAY_2, AluInp.PREV_DELAY_2)
    dp[1].pass_through_delay(2)
    dp[2].pass_through_alu()
    dp[2].pass_through_delay(2)
    dp[3].enable_alu(AluOp.BYPASS, AluInp.PREV_DELAY_2, AluInp.PREV_DELAY_2)
    for b in range(4, 8):
        dp[b].pass_through_alu()
    u.repeat_count = 1
    u.trigger = (Trigger.COUNT, Trigger.NONE, Trigger.NONE)
    u.next_uop = (1, 0, 0)
    return u


def _prodscan2x_steady():
    """2x steady for scan(ADD, Src0*Src1): per-pair running total streamed
    to both write halves; with a stride-0-broadcast out AP the final word
    is the full row sum."""
    u = UopConfig()
    u.enable_input(InpSel.SRC_0, 1)     # -> d0
    u.enable_input(InpSel.SRC_1, 2)     # -> d1
    u.enable_input(InpSel.SRC_0_HI, 4)  # -> d3
    u.enable_input(InpSel.SRC_1_HI, 5)  # -> d4
    dp = u.datapath_config
    dp[0].enable_alu(AluOp.MULTIPLY, AluInp.PREV_DELAY_0, AluInp.PREV_DELAY_1)
    dp[0].pass_through_delay(3, 4)
    dp[1].enable_alu(AluOp.ADD, AluInp.CURR_ALU_OUT, AluInp.PREV_ALU_OUT)
    dp[1].pass_through_delay(3, 4)
    dp[2].enable_alu(AluOp.MULTIPLY, AluInp.PREV_DELAY_3, AluInp.PREV_DELAY_4)
    dp[2].enable_delay_from_src(DelayInp.PREV_ALU_OUT, 5)
    dp[3].enable_alu(AluOp.ADD, AluInp.CURR_ALU_OUT, AluInp.PREV_ALU_OUT)
    dp[3].pass_through_delay(5)
    dp[4].enable_alu(AluOp.ADD, AluInp.PREV_ALU_OUT, AluInp.PREV_DELAY_5)
    for b in range(5, 8):
        dp[b].pass_through_alu()
    u.enable_output(OutSel.ALU_OUT, OutPath.WR0_LO)
    u.enable_output(OutSel.ALU_OUT, OutPath.WR0_HI)
    u.require_inp0 = ENABLE
    u.require_inp1 = ENABLE
    u.trigger = (Trigger.SRC_TENSOR_DONE, Trigger.NONE, Trigger.NONE)
    u.next_uop = (0, 0, 0)
    return u


def _register(name, spec, steady_2x, seed_2x=None):
    """Register a DveOp whose compiled DveOpSpec carries a 2x program, by
    pre-populating dve_ops._COMPILE_CACHE (compile() returns the cache hit
    before its lower()-drift sha check)."""
    for op in dve_ops.OPS:
        if op.name == name:
            return op
    row = dve_ops._CUSTOM_DVE_ROW_BASE + len(dve_ops.OPS)
    uops_1x = lower(spec, ver="v3")
    uops_2x = [seed_2x if seed_2x is not None else copy.deepcopy(uops_1x[0]),
               steady_2x]
    assert len(uops_2x) == len(uops_1x)
    compiled = DveOpSpec(
        name=name,
        opcode=row,
        uops=uops_1x,
        uops_2x=uops_2x,
        perf_max=1,  # byte-36[7:6]: highest reachable slot = 2X_1PORT
        rd1_en=_has_src1(spec),
    )
    compiled.validate("v3")
    shas = {"v3": compiled.sha("v3")}
    op = dve_ops.DveOp(name, spec, subdim=False, uops_sha=shas,
                       perf_en={"v3": True})
    dve_ops.OPS.append(op)
    dve_ops.CUSTOM_DVE_SPECS[name] = spec
    dve_ops._SUB_OPCODE_FOR_NAME[name] = row
    dve_ops._COMPILE_CACHE[(name, "v3")] = compiled
    return op


def _get_lerp2x():
    spec = Spec(
        body=(Src0 - Src1) * (C2 - C0 * C1) + Src1,
        reference=lambda in0, in1, s0, s1, imm2: (in0.astype(np.float32) - in1)
        * (imm2 - s0 * s1) + in1,
    )
    return _register("LERP2X_ANT", spec, _lerp2x_steady())


def _get_prodscan2x():
    spec = Spec(
        body=scan(SAluOp.ADD, Src0 * Src1),
        reference=lambda in0, in1, s0, s1, imm2: np.cumsum(
            in0.astype(np.float32) * in1, axis=-1, dtype=np.float32),
    )
    return _register("PRODSCAN2X_ANT", spec, _prodscan2x_steady(),
                     _prodscan2x_seed())


def _emit2x(nc, op, *, out, in0, in1, s0=0.0, s1=0.0, imm2=0.0):
    bi = nc.vector._custom_dve(
        op, out=out, in0=in0, in1=in1, s0=s0, s1=s1, imm2=imm2)
    bi.ins.perf_max = 1
    return bi


# ---------------------------------------------------------------------------
# Kernel
# ---------------------------------------------------------------------------

# Unit schedule: (rpp, lo, hi, ssb_on_dve) slices of the rpp-grouped row
# view. A group covers rpp*128 consecutive rows; partition p holds rpp
# consecutive rows concatenated along the free dim (rpp*2KB contiguous DMA
# lines at bf16). t = (hi-lo)*rpp sub-tiles of [128, 1024] per unit.
UNITS = [
    (1, 0, 1, True),    # rows 0-127     t=1  (small first units: compute
    (1, 1, 2, True),    # rows 128-255   t=1   starts as soon as 256KB lands)
    (2, 1, 2, True),    # rows 256-511   t=2  ssB on DVE
    (2, 2, 4, True),    # rows 512-1023  t=4  ssB on DVE
    (2, 4, 6, False),   # rows 1024-1535 t=4  ssB on ACT
    (2, 6, 7, False),   # rows 1536-1791 t=2  ssB on ACT
    (1, 14, 15, True),  # rows 1792-1919 t=1  ssB on DVE
    (1, 15, 16, True),  # rows 1920-2047 t=1  ssB on DVE (short final chain)
]


class _Chain:
    """Per-engine self-chain. Every op increments the chain sem (so
    cross-engine marks stay meaningful); the wait on the previous op is
    only emitted when the caller says there is a same-engine RAW/WAW hazard
    (ops on disjoint tiles issue in order anyway and the ~1-op pipeline
    overlap cannot reorder disjoint accesses)."""

    def __init__(self, nc, eng, name):
        self.eng = eng
        self.sem = nc.alloc_semaphore(name)
        self.n = 0

    def emit(self, fn, wait=True):
        if wait and self.n > 0:
            self.eng.wait_ge(self.sem, self.n)
        inst = fn()
        inst.then_inc(self.sem, 1)
        self.n += 1
        return inst


def build_program():
    Sq = mybir.ActivationFunctionType.Square
    Sqrt = mybir.ActivationFunctionType.Sqrt
    lerp2x = _get_lerp2x()
    pscan = _get_prodscan2x()

    nc = bacc.Bacc()
    A = nc.declare_dram_parameter("A", [ROWS, D], BF16, isOutput=False)
    B = nc.declare_dram_parameter("B", [ROWS, D], BF16, isOutput=False)
    O = nc.declare_dram_parameter("out", [ROWS, D], BF16, isOutput=True)

    def views(T):
        return {
            r: T[:].rearrange("(g p r) d -> g p (r d)", p=P, r=r)
            for r in (1, 2)
        }

    Av, Bv, Ov = views(A), views(B), views(O)

    def dram_ap(vs, u):
        rpp, lo, hi = u[:3]
        return vs[rpp][lo:hi].rearrange("g p f -> p g f")

    n_units = len(UNITS)
    ts = [(hi - lo) * rpp for rpp, lo, hi, _ in UNITS]
    assert sum(ts) == ROWS // P
    rows = []
    for rpp, lo, hi, _ in UNITS:
        rows.extend(range(lo * rpp * P, hi * rpp * P))
    assert sorted(rows) == list(range(ROWS))

    a_tiles, b_tiles = [], []
    for i in range(n_units):
        shape = [P, ts[i] * D]
        a_tiles.append(nc.alloc_sbuf_tensor(f"a{i}", shape, BF16))
        b_tiles.append(nc.alloc_sbuf_tensor(f"b{i}", shape, BF16))

    def sub_ap(tile, j):
        return tile[:, j * D : (j + 1) * D]

    # Reduction slot tiles (bf16, 2 cols per subtile; col 1 = the total) for
    # DVE prodscan results; f32 accum tiles for ACT square-accum results.
    dslot_t, sbslot_t = [], []
    ssA_t, ssB_t = [], []
    dotf_t, sc_t, sa_t, sb_t, r_t = [], [], [], [], []
    for i, t in enumerate(ts):
        dslot_t.append(nc.alloc_sbuf_tensor(f"dsl{i}", [P, 2 * t], BF16))
        sbslot_t.append(nc.alloc_sbuf_tensor(f"bsl{i}", [P, 2 * t], BF16))
        ssA_t.append(nc.alloc_sbuf_tensor(f"ssA{i}", [P, t], F32))
        ssB_t.append(nc.alloc_sbuf_tensor(f"ssB{i}", [P, t], F32))
        dotf_t.append(nc.alloc_sbuf_tensor(f"dotf{i}", [P, t], F32))
        sc_t.append(nc.alloc_sbuf_tensor(f"sc{i}", [P, t], F32))
        sa_t.append(nc.alloc_sbuf_tensor(f"sa{i}", [P, t], F32))
        sb_t.append(nc.alloc_sbuf_tensor(f"sb{i}", [P, t], F32))
        r_t.append(nc.alloc_sbuf_tensor(f"r{i}", [P, t], F32))

    # two ACT dump tiles, alternated so consecutive square ops never share a
    # dump (lets them run without intra-unit chain waits)
    adumps = [nc.alloc_sbuf_tensor("adump0", [P, D], BF16),
              nc.alloc_sbuf_tensor("adump1", [P, D], BF16)]
    # last unit's ssA also runs on DVE (prodscan A*A) so its lerp+store
    # chain only waits on ACT for two tiny sqrts
    aslotL = nc.alloc_sbuf_tensor("aslotL", [P, 2], BF16)

    ldA = [nc.alloc_semaphore(f"ldA{i}") for i in range(n_units)]
    ldB = [nc.alloc_semaphore(f"ldB{i}") for i in range(n_units)]
    stS = [nc.alloc_semaphore(f"st{i}") for i in range(n_units)]

    act_ch = _Chain(nc, nc.scalar, "act_ch")
    dve_ch = _Chain(nc, nc.vector, "dve_ch")

    def scan_out(slot_tile, j):
        # [P,2] slot broadcast to [P,512,2]: every pair-write lands on the
        # same word; the last write is the full row sum.
        return slot_tile[:, 2 * j : 2 * j + 2].rearrange(
            "p (o i) -> p o i", o=1).broadcast_to((P, D // 2, 2))

    def slot_hi(slot_tile, t):
        # strided [P, t] view of the totals (col 1 of each pair)
        return slot_tile[:].rearrange("p (t i) -> p t i", i=2)[:, :, 1:2]

    # ---- Loads: A tiles on the SP HWDGE ring, B tiles on the gpsimd SWDGE
    # ring (idle until the store phase). Parallel descriptor supply and each
    # stream arrives ~2x sooner than the single-ring A/B interleave, so ACT
    # (A-only early) and DVE (A+B) stop stalling on mid-stream tiles.
    for i, u in enumerate(UNITS):
        nc.sync.dma_start(a_tiles[i][:], dram_ap(Av, u)).then_inc(ldA[i], 16)
    for i, u in enumerate(UNITS):
        nc.gpsimd.dma_start(b_tiles[i][:], dram_ap(Bv, u)).then_inc(ldB[i], 16)

    act_sq_marks = [0] * n_units    # act_ch count after unit i's squares+sqrts
    dve_red_marks = [0] * n_units   # dve_ch count after unit i's reductions
    dve_lerp_marks = [0] * n_units  # dve_ch count after unit i's last lerp

    def emit_act_squares(i):
        # Squares, then the unit's sqrts immediately (ACT-local; for DVE-ssB
        # units the sqrt reads the DVE slot totals -- that wait is satisfied
        # long before ACT reaches it, since DVE reductions run ~2x faster).
        t = ts[i]
        dve_ssb = UNITS[i][3]
        dve_ssa = i == n_units - 1
        nsq = 0
        if not dve_ssa:
            nc.scalar.wait_ge(ldA[i], 16)
            for j in range(t):
                act_ch.emit(lambda i=i, j=j, n=nsq: nc.scalar.activation(
                    adumps[n % 2][:], sub_ap(a_tiles[i], j), Sq,
                    accum_out=ssA_t[i][:, j : j + 1],
                ), wait=nsq == 0)
                nsq += 1
        if not dve_ssb:
            nc.scalar.wait_ge(ldB[i], 16)
            for j in range(t):
                act_ch.emit(lambda i=i, j=j, n=nsq: nc.scalar.activation(
                    adumps[n % 2][:], sub_ap(b_tiles[i], j), Sq,
                    accum_out=ssB_t[i][:, j : j + 1],
                ), wait=nsq == 0)
                nsq += 1
        # sa = sqrt(2*ssA), sb = sqrt(2*ssB): sa*sb = 2|A||B|
        if dve_ssa or dve_ssb:
            nc.scalar.wait_ge(dve_ch.sem, dve_red_marks[i])
        if dve_ssa:
            act_ch.emit(lambda i=i: nc.scalar.activation(
                sa_t[i][:], slot_hi(aslotL, 1), Sqrt, scale=2.0))
        else:
            act_ch.emit(lambda i=i: nc.scalar.activation(
                sa_t[i][:], ssA_t[i][:], Sqrt, scale=2.0))
        if dve_ssb:
            act_ch.emit(lambda i=i, t=t: nc.scalar.activation(
                sb_t[i][:], slot_hi(sbslot_t[i], t), Sqrt, scale=2.0))
        else:
            act_ch.emit(lambda i=i: nc.scalar.activation(
                sb_t[i][:], ssB_t[i][:], Sqrt, scale=2.0))
        act_sq_marks[i] = act_ch.n

    def emit_dve_reductions(i):
        # scans write disjoint slot words -- no intra-unit chain waits
        t = ts[i]
        nc.vector.wait_ge(ldA[i], 16)
        nc.vector.wait_ge(ldB[i], 16)
        ns = 0
        for j in range(t):
            dve_ch.emit(lambda i=i, j=j: _emit2x(
                nc, pscan, out=scan_out(dslot_t[i], j),
                in0=sub_ap(a_tiles[i], j), in1=sub_ap(b_tiles[i], j)),
                wait=ns == 0)
            ns += 1
        if UNITS[i][3]:
            for j in range(t):
                dve_ch.emit(lambda i=i, j=j: _emit2x(
                    nc, pscan, out=scan_out(sbslot_t[i], j),
                    in0=sub_ap(b_tiles[i], j), in1=sub_ap(b_tiles[i], j)),
                    wait=ns == 0)
                ns += 1
        if i == n_units - 1:
            dve_ch.emit(lambda i=i: _emit2x(
                nc, pscan, out=scan_out(aslotL, 0),
                in0=sub_ap(a_tiles[i], 0), in1=sub_ap(a_tiles[i], 0)),
                wait=False)
        dve_red_marks[i] = dve_ch.n

    def emit_dve_finish(i):
        t = ts[i]
        # dot -> f32 (for the lerp scalar operand); DVE-local
        dve_ch.emit(lambda i=i, t=t: nc.vector.tensor_copy(
            dotf_t[i][:], slot_hi(dslot_t[i], t)))
        # s = sa*sb = 2|A||B| (waits once on ACT's squares+sqrts), clamp, r=1/s
        nc.vector.wait_ge(act_ch.sem, act_sq_marks[i])
        dve_ch.emit(lambda i=i: nc.vector.tensor_mul(
            sc_t[i][:], sa_t[i][:], sb_t[i][:]))
        dve_ch.emit(lambda i=i: nc.vector.tensor_scalar_max(
            sc_t[i][:], sc_t[i][:], 2 * EPS))
        dve_ch.emit(lambda i=i: nc.vector.reciprocal(r_t[i][:], sc_t[i][:]))
        # lerps write disjoint subtiles; only the first needs the chain wait
        # (RAW on r_t from the reciprocal)
        for j in range(t):
            dve_ch.emit(lambda i=i, j=j: _emit2x(
                nc, lerp2x,
                out=sub_ap(b_tiles[i], j),
                in0=sub_ap(a_tiles[i], j),
                in1=sub_ap(b_tiles[i], j),
                s0=dotf_t[i][:, j : j + 1],
                s1=r_t[i][:, j : j + 1],
                imm2=0.5,
            ), wait=j == 0)
        dve_lerp_marks[i] = dve_ch.n

    # Emission interleave: DVE runs one unit ahead on reductions; stats and
    # lerps of unit i-1 are emitted after the reductions of unit i.
    for i in range(n_units):
        emit_dve_reductions(i)
        emit_act_squares(i)
        if i > 0:
            emit_dve_finish(i - 1)
    emit_dve_finish(n_units - 1)

    # ---- Stores, deferred until ALL loads are done (no HBM r/w mixing).
    # Bulk units stream on the SWDGE ring; the last two (tiny, tail-critical)
    # units go on the HWDGE ring (idle after the A-loads, ~0.4us lower
    # first-byte latency).
    nc.gpsimd.wait_ge(ldA[n_units - 1], 16)
    nc.gpsimd.wait_ge(ldB[n_units - 1], 16)
    for i, u in enumerate(UNITS[: n_units - 2]):
        nc.gpsimd.wait_ge(dve_ch.sem, dve_lerp_marks[i])
        nc.gpsimd.dma_start(dram_ap(Ov, u), b_tiles[i][:]).then_inc(stS[i], 16)
    for i in (n_units - 2, n_units - 1):
        nc.sync.wait_ge(dve_ch.sem, dve_lerp_marks[i])
        nc.sync.dma_start(
            dram_ap(Ov, UNITS[i]), b_tiles[i][:]).then_inc(stS[i], 16)

    # ---- SP tail: program is done when every store has landed ----
    for i in range(n_units):
        nc.sync.wait_ge(stS[i], 16)

    nc.finalize()
    return nc


_prog_cache = {}


def _get_program():
    key = ("v4", ROWS, D)
    if key not in _prog_cache:
        _prog_cache[key] = build_program()
    return _prog_cache[key]


def make_in_maps(A, B):
    """Shard full f32 inputs row-wise into per-core bf16 in_maps."""
    A = np.asarray(A, dtype=np.float32).astype(NP_BF16)
    B = np.asarray(B, dtype=np.float32).astype(NP_BF16)
    assert A.shape == (N_FULL, D) and B.shape == (N_FULL, D)
    return [
        {
            "A": np.ascontiguousarray(A[i * ROWS : (i + 1) * ROWS]),
            "B": np.ascontiguousarray(B[i * ROWS : (i + 1) * ROWS]),
        }
        for i in range(NCORES)
    ]


def kernel(A, B):
    from concourse.bass_utils import run_bass_kernel_spmd

    nc = _get_program()
    in_maps = make_in_maps(A, B)
    res = run_bass_kernel_spmd(nc, in_maps, list(range(NCORES)))
    return np.concatenate(
        [np.asarray(res.results[i]["out"]).astype(np.float32) for i in range(NCORES)],
        axis=0,
    )


# revision 39
# speedup vs baseline: 1.0215x; 1.0215x over previous
"""Manual-sync (raw bacc) Trainium2 kernel for fused cosine-distance row merge.

Per row i: out[i] = u_i*A[i] + (1-u_i)*B[i], u_i = 0.5 - 0.5*dot_i/scale_i,
scale_i = max(|A_i||B_i|, 1e-8).

bf16 I/O: host converts f32 inputs to bf16 (rel-err ~2e-3, well under the
2e-2 gate), halving HBM traffic: 8MB loads + 4MB stores per core.

Custom DVE ops with hand-authored 2X_1PORT uop programs (the stock lower()
only emits the 1x program; hardware-measured per [128,1024] bf16 subtile):
  - PRODSCAN2X: prefix-scan of Src0*Src1. The out AP is a [P,2] slot
    broadcast to [P,512,2] (outer stride 0), so the final written word is
    the full row dot/sum-of-squares in bf16. ~692ns vs 1305ns for the 1x
    scalar_tensor_tensor+accum path. Used for dots and for ssB on part of
    the units.
  - LERP2X: (in0-in1)*(imm2-s0*s1)+in1 at 2x: ~811ns vs 1344ns.
Both ops also carry the stock-lowered 1x program, so a silent perf-mode
fallback still computes correct values.

Engine assignment (balanced ~33us each, at the ~33.5us/core DMA floor):
  - DVE: all 16 dots, ssB for units 0,1,4,5 (10 subtiles), all 16 lerps,
    per-unit stats (dot->f32 copy, sc=ssA*ssB, clamp, reciprocal).
  - ACT: ssA for all units + ssB for units 2,3 (square activation with
    f32 accum), per-unit sqrt.
  - DVE runs one unit ahead on dots/ssB; stats+lerps of unit i-1 are
    emitted after the dots of unit i so DVE never stalls on ACT.
  - Loads: HWDGE (SP) back-to-back from t=0. Stores: SWDGE (gpsimd) after
    all loads (avoids chip-wide HBM read/write mixing), per-unit once that
    unit's lerps retire.
"""

import copy

import ml_dtypes
import numpy as np

import concourse.bacc as bacc
import concourse.mybir as mybir
from concourse import dve_ops
from concourse.dve_spec import (
    Spec, Src0, Src1, C0, C1, C2, lower, _has_src1, scan, AluOp as SAluOp,
)
from concourse.dve_uop import (
    DveOpSpec, UopConfig, InpSel, OutSel, OutPath, AluInp, DelayInp,
    Trigger, AluOp, ENABLE,
)

N_FULL = 16384
D = 1024
NCORES = 8
ROWS = N_FULL // NCORES  # 2048
P = 128
EPS = 1e-8

F32 = mybir.dt.float32
BF16 = mybir.dt.bfloat16
NP_BF16 = ml_dtypes.bfloat16


# ---------------------------------------------------------------------------
# Custom DVE ops with 2X_1PORT programs
# ---------------------------------------------------------------------------

def _lerp2x_steady():
    """2x steady: lo result rides delay chain 0 to the end, hi at ALU out."""
    u = UopConfig()
    u.enable_input(InpSel.SRC_0, 1)     # -> d0
    u.enable_input(InpSel.SRC_1, 2)     # -> d1
    u.enable_input(InpSel.CONST_2, 3)   # -> d2
    u.enable_input(InpSel.SRC_0_HI, 4)  # -> d3
    u.enable_input(InpSel.SRC_1_HI, 5)  # -> d4
    dp = u.datapath_config
    dp[0].enable_alu(AluOp.SUBTRACT, AluInp.PREV_DELAY_0, AluInp.PREV_DELAY_1)
    dp[0].pass_through_delay(1, 2, 3, 4)
    # w = imm2 - s0*s1 (s0*s1 latched into this slice's swap flop by uop0)
    dp[1].enable_alu(AluOp.SUBTRACT, AluInp.PREV_DELAY_2, AluInp.CURR_SWAP_OUT)
    dp[1].enable_delay_from_src(DelayInp.PREV_ALU_OUT, 0)
    dp[1].pass_through_delay(1, 3, 4)
    dp[2].enable_alu(AluOp.MULTIPLY, AluInp.PREV_DELAY_0, AluInp.PREV_ALU_OUT)
    dp[2].enable_delay_from_src(DelayInp.PREV_ALU_OUT, 2)
    dp[2].pass_through_delay(1, 3, 4)
    dp[3].enable_alu(AluOp.ADD, AluInp.PREV_ALU_OUT, AluInp.PREV_DELAY_1)
    dp[3].pass_through_delay(2, 3, 4)
    dp[4].enable_alu(AluOp.SUBTRACT, AluInp.PREV_DELAY_3, AluInp.PREV_DELAY_4)
    dp[4].enable_delay_from_src(DelayInp.PREV_ALU_OUT, 0)
    dp[4].pass_through_delay(2, 4)
    dp[5].enable_alu(AluOp.MULTIPLY, AluInp.PREV_ALU_OUT, AluInp.PREV_DELAY_2)
    dp[5].pass_through_delay(0, 4)
    dp[6].enable_alu(AluOp.ADD, AluInp.PREV_ALU_OUT, AluInp.PREV_DELAY_4)
    dp[6].pass_through_delay(0)
    dp[7].pass_through_alu()
    dp[7].pass_through_delay(0)
    u.enable_output(OutSel.DELAY_0, OutPath.WR0_LO)
    u.enable_output(OutSel.ALU_OUT, OutPath.WR0_HI)
    u.require_inp0 = ENABLE
    u.require_inp1 = ENABLE
    u.trigger = (Trigger.SRC_TENSOR_DONE, Trigger.NONE, Trigger.NONE)
    u.next_uop = (0, 0, 0)
    return u


def _prodscan2x_seed():
    """Runs once before the stream (consumes nothing): zero the feedback
    flops at blk1 (lo partial sum) and blk3 (hi partial sum)."""
    u = UopConfig()
    u.enable_input(InpSel.ZERO, 3)  # -> d2
    dp = u.datapath_config
    dp[0].pass_through_delay(2)
    dp[1].enable_alu(AluOp.BYPASS, AluInp.PREV_DELAY_2, AluInp.PREV_DELAY_2)
    dp[1].pass_through_delay(2)
    dp[2].pass_through_alu()
    dp[2].pass_through_delay(2)
    dp[3].enable_alu(AluOp.BYPASS, AluInp.PREV_DELAY_2, AluInp.PREV_DELAY_2)
    for b in range(4, 8):
        dp[b].pass_through_alu()
    u.repeat_count = 1
    u.trigger = (Trigger.COUNT, Trigger.NONE, Trigger.NONE)
    u.next_uop = (1, 0, 0)
    return u


def _prodscan2x_steady():
    """2x steady for scan(ADD, Src0*Src1): per-pair running total streamed
    to both write halves; with a stride-0-broadcast out AP the final word
    is the full row sum."""
    u = UopConfig()
    u.enable_input(InpSel.SRC_0, 1)     # -> d0
    u.enable_input(InpSel.SRC_1, 2)     # -> d1
    u.enable_input(InpSel.SRC_0_HI, 4)  # -> d3
    u.enable_input(InpSel.SRC_1_HI, 5)  # -> d4
    dp = u.datapath_config
    dp[0].enable_alu(AluOp.MULTIPLY, AluInp.PREV_DELAY_0, AluInp.PREV_DELAY_1)
    dp[0].pass_through_delay(3, 4)
    dp[1].enable_alu(AluOp.ADD, AluInp.CURR_ALU_OUT, AluInp.PREV_ALU_OUT)
    dp[1].pass_through_delay(3, 4)
    dp[2].enable_alu(AluOp.MULTIPLY, AluInp.PREV_DELAY_3, AluInp.PREV_DELAY_4)
    dp[2].enable_delay_from_src(DelayInp.PREV_ALU_OUT, 5)
    dp[3].enable_alu(AluOp.ADD, AluInp.CURR_ALU_OUT, AluInp.PREV_ALU_OUT)
    dp[3].pass_through_delay(5)
    dp[4].enable_alu(AluOp.ADD, AluInp.PREV_ALU_OUT, AluInp.PREV_DELAY_5)
    for b in range(5, 8):
        dp[b].pass_through_alu()
    u.enable_output(OutSel.ALU_OUT, OutPath.WR0_LO)
    u.enable_output(OutSel.ALU_OUT, OutPath.WR0_HI)
    u.require_inp0 = ENABLE
    u.require_inp1 = ENABLE
    u.trigger = (Trigger.SRC_TENSOR_DONE, Trigger.NONE, Trigger.NONE)
    u.next_uop = (0, 0, 0)
    return u


def _register(name, spec, steady_2x, seed_2x=None):
    """Register a DveOp whose compiled DveOpSpec carries a 2x program, by
    pre-populating dve_ops._COMPILE_CACHE (compile() returns the cache hit
    before its lower()-drift sha check)."""
    for op in dve_ops.OPS:
        if op.name == name:
            return op
    row = dve_ops._CUSTOM_DVE_ROW_BASE + len(dve_ops.OPS)
    uops_1x = lower(spec, ver="v3")
    uops_2x = [seed_2x if seed_2x is not None else copy.deepcopy(uops_1x[0]),
               steady_2x]
    assert len(uops_2x) == len(uops_1x)
    compiled = DveOpSpec(
        name=name,
        opcode=row,
        uops=uops_1x,
        uops_2x=uops_2x,
        perf_max=1,  # byte-36[7:6]: highest reachable slot = 2X_1PORT
        rd1_en=_has_src1(spec),
    )
    compiled.validate("v3")
    shas = {"v3": compiled.sha("v3")}
    op = dve_ops.DveOp(name, spec, subdim=False, uops_sha=shas,
                       perf_en={"v3": True})
    dve_ops.OPS.append(op)
    dve_ops.CUSTOM_DVE_SPECS[name] = spec
    dve_ops._SUB_OPCODE_FOR_NAME[name] = row
    dve_ops._COMPILE_CACHE[(name, "v3")] = compiled
    return op


def _get_lerp2x():
    spec = Spec(
        body=(Src0 - Src1) * (C2 - C0 * C1) + Src1,
        reference=lambda in0, in1, s0, s1, imm2: (in0.astype(np.float32) - in1)
        * (imm2 - s0 * s1) + in1,
    )
    return _register("LERP2X_ANT", spec, _lerp2x_steady())


def _get_prodscan2x():
    spec = Spec(
        body=scan(SAluOp.ADD, Src0 * Src1),
        reference=lambda in0, in1, s0, s1, imm2: np.cumsum(
            in0.astype(np.float32) * in1, axis=-1, dtype=np.float32),
    )
    return _register("PRODSCAN2X_ANT", spec, _prodscan2x_steady(),
                     _prodscan2x_seed())


def _emit2x(nc, op, *, out, in0, in1, s0=0.0, s1=0.0, imm2=0.0):
    bi = nc.vector._custom_dve(
        op, out=out, in0=in0, in1=in1, s0=s0, s1=s1, imm2=imm2)
    bi.ins.perf_max = 1
    return bi


# ---------------------------------------------------------------------------
# Kernel
# ---------------------------------------------------------------------------

# Unit schedule: (rpp, lo, hi, ssb_on_dve) slices of the rpp-grouped row
# view. A group covers rpp*128 consecutive rows; partition p holds rpp
# consecutive rows concatenated along the free dim (rpp*2KB contiguous DMA
# lines at bf16). t = (hi-lo)*rpp sub-tiles of [128, 1024] per unit.
UNITS = [
    (1, 0, 1, True),    # rows 0-127     t=1  (small first units: compute
    (1, 1, 2, True),    # rows 128-255   t=1   starts as soon as 256KB lands)
    (2, 1, 2, True),    # rows 256-511   t=2  ssB on DVE
    (2, 2, 4, True),    # rows 512-1023  t=4  ssB on DVE
    (2, 4, 6, False),   # rows 1024-1535 t=4  ssB on ACT
    (2, 6, 7, False),   # rows 1536-1791 t=2  ssB on ACT
    (1, 14, 15, True),  # rows 1792-1919 t=1  ssB on DVE
    (1, 15, 16, True),  # rows 1920-2047 t=1  ssB on DVE (short final chain)
]


class _Chain:
    """Per-engine self-chain. Every op increments the chain sem (so
    cross-engine marks stay meaningful); the wait on the previous op is
    only emitted when the caller says there is a same-engine RAW/WAW hazard
    (ops on disjoint tiles issue in order anyway and the ~1-op pipeline
    overlap cannot reorder disjoint accesses)."""

    def __init__(self, nc, eng, name):
        self.eng = eng
        self.sem = nc.alloc_semaphore(name)
        self.n = 0

    def emit(self, fn, wait=True):
        if wait and self.n > 0:
            self.eng.wait_ge(self.sem, self.n)
        inst = fn()
        inst.then_inc(self.sem, 1)
        self.n += 1
        return inst


def build_program():
    Sq = mybir.ActivationFunctionType.Square
    Sqrt = mybir.ActivationFunctionType.Sqrt
    lerp2x = _get_lerp2x()
    pscan = _get_prodscan2x()

    nc = bacc.Bacc()
    A = nc.declare_dram_parameter("A", [ROWS, D], BF16, isOutput=False)
    B = nc.declare_dram_parameter("B", [ROWS, D], BF16, isOutput=False)
    O = nc.declare_dram_parameter("out", [ROWS, D], BF16, isOutput=True)

    def views(T):
        return {
            r: T[:].rearrange("(g p r) d -> g p (r d)", p=P, r=r)
            for r in (1, 2)
        }

    Av, Bv, Ov = views(A), views(B), views(O)

    def dram_ap(vs, u):
        rpp, lo, hi = u[:3]
        return vs[rpp][lo:hi].rearrange("g p f -> p g f")

    n_units = len(UNITS)
    ts = [(hi - lo) * rpp for rpp, lo, hi, _ in UNITS]
    assert sum(ts) == ROWS // P
    rows = []
    for rpp, lo, hi, _ in UNITS:
        rows.extend(range(lo * rpp * P, hi * rpp * P))
    assert sorted(rows) == list(range(ROWS))

    a_tiles, b_tiles = [], []
    for i in range(n_units):
        shape = [P, ts[i] * D]
        a_tiles.append(nc.alloc_sbuf_tensor(f"a{i}", shape, BF16))
        b_tiles.append(nc.alloc_sbuf_tensor(f"b{i}", shape, BF16))

    def sub_ap(tile, j):
        return tile[:, j * D : (j + 1) * D]

    # Reduction slot tiles (bf16, 2 cols per subtile; col 1 = the total) for
    # DVE prodscan results; f32 accum tiles for ACT square-accum results.
    dslot_t, sbslot_t = [], []
    ssA_t, ssB_t = [], []
    dotf_t, sc_t, sa_t, sb_t, r_t = [], [], [], [], []
    for i, t in enumerate(ts):
        dslot_t.append(nc.alloc_sbuf_tensor(f"dsl{i}", [P, 2 * t], BF16))
        sbslot_t.append(nc.alloc_sbuf_tensor(f"bsl{i}", [P, 2 * t], BF16))
        ssA_t.append(nc.alloc_sbuf_tensor(f"ssA{i}", [P, t], F32))
        ssB_t.append(nc.alloc_sbuf_tensor(f"ssB{i}", [P, t], F32))
        dotf_t.append(nc.alloc_sbuf_tensor(f"dotf{i}", [P, t], F32))
        sc_t.append(nc.alloc_sbuf_tensor(f"sc{i}", [P, t], F32))
        sa_t.append(nc.alloc_sbuf_tensor(f"sa{i}", [P, t], F32))
        sb_t.append(nc.alloc_sbuf_tensor(f"sb{i}", [P, t], F32))
        r_t.append(nc.alloc_sbuf_tensor(f"r{i}", [P, t], F32))

    # two ACT dump tiles, alternated so consecutive square ops never share a
    # dump (lets them run without intra-unit chain waits)
    adumps = [nc.alloc_sbuf_tensor("adump0", [P, D], BF16),
              nc.alloc_sbuf_tensor("adump1", [P, D], BF16)]
    # last unit's ssA also runs on DVE (prodscan A*A) so its lerp+store
    # chain only waits on ACT for two tiny sqrts
    aslotL = nc.alloc_sbuf_tensor("aslotL", [P, 2], BF16)

    ldA = [nc.alloc_semaphore(f"ldA{i}") for i in range(n_units)]
    ldB = [nc.alloc_semaphore(f"ldB{i}") for i in range(n_units)]
    stS = [nc.alloc_semaphore(f"st{i}") for i in range(n_units)]

    act_ch = _Chain(nc, nc.scalar, "act_ch")
    dve_ch = _Chain(nc, nc.vector, "dve_ch")

    def scan_out(slot_tile, j):
        # [P,2] slot broadcast to [P,512,2]: every pair-write lands on the
        # same word; the last write is the full row sum.
        return slot_tile[:, 2 * j : 2 * j + 2].rearrange(
            "p (o i) -> p o i", o=1).broadcast_to((P, D // 2, 2))

    def slot_hi(slot_tile, t):
        # strided [P, t] view of the totals (col 1 of each pair)
        return slot_tile[:].rearrange("p (t i) -> p t i", i=2)[:, :, 1:2]

    # ---- Loads: A tiles on the SP HWDGE ring, B tiles on the gpsimd SWDGE
    # ring (idle until the store phase). Parallel descriptor supply and each
    # stream arrives ~2x sooner than the single-ring A/B interleave, so ACT
    # (A-only early) and DVE (A+B) stop stalling on mid-stream tiles.
    for i, u in enumerate(UNITS):
        nc.sync.dma_start(a_tiles[i][:], dram_ap(Av, u)).then_inc(ldA[i], 16)
    for i, u in enumerate(UNITS):
        nc.gpsimd.dma_start(b_tiles[i][:], dram_ap(Bv, u)).then_inc(ldB[i], 16)

    act_sq_marks = [0] * n_units    # act_ch count after unit i's squares+sqrts
    dve_red_marks = [0] * n_units   # dve_ch count after unit i's reductions
    dve_lerp_marks = [0] * n_units  # dve_ch count after unit i's last lerp

    def emit_act_squares(i):
        # Squares, then the unit's sqrts immediately (ACT-local; for DVE-ssB
        # units the sqrt reads the DVE slot totals -- that wait is satisfied
        # long before ACT reaches it, since DVE reductions run ~2x faster).
        t = ts[i]
        dve_ssb = UNITS[i][3]
        dve_ssa = i == n_units - 1
        nsq = 0
        if not dve_ssa:
            nc.scalar.wait_ge(ldA[i], 16)
            for j in range(t):
                act_ch.emit(lambda i=i, j=j, n=nsq: nc.scalar.activation(
                    adumps[n % 2][:], sub_ap(a_tiles[i], j), Sq,
                    accum_out=ssA_t[i][:, j : j + 1],
                ), wait=nsq == 0)
                nsq += 1
        if not dve_ssb:
            nc.scalar.wait_ge(ldB[i], 16)
            for j in range(t):
                act_ch.emit(lambda i=i, j=j, n=nsq: nc.scalar.activation(
                    adumps[n % 2][:], sub_ap(b_tiles[i], j), Sq,
                    accum_out=ssB_t[i][:, j : j + 1],
                ), wait=nsq == 0)
                nsq += 1
        # sa = sqrt(2*ssA), sb = sqrt(2*ssB): sa*sb = 2|A||B|
        if dve_ssa or dve_ssb:
            nc.scalar.wait_ge(dve_ch.sem, dve_red_marks[i])
        if dve_ssa:
            act_ch.emit(lambda i=i: nc.scalar.activation(
                sa_t[i][:], slot_hi(aslotL, 1), Sqrt, scale=2.0))
        else:
            act_ch.emit(lambda i=i: nc.scalar.activation(
                sa_t[i][:], ssA_t[i][:], Sqrt, scale=2.0))
        if dve_ssb:
            act_ch.emit(lambda i=i, t=t: nc.scalar.activation(
                sb_t[i][:], slot_hi(sbslot_t[i], t), Sqrt, scale=2.0))
        else:
            act_ch.emit(lambda i=i: nc.scalar.activation(
                sb_t[i][:], ssB_t[i][:], Sqrt, scale=2.0))
        act_sq_marks[i] = act_ch.n

    def emit_dve_reductions(i):
        # scans write disjoint slot words -- no intra-unit chain waits
        t = ts[i]
        nc.vector.wait_ge(ldA[i], 16)
        nc.vector.wait_ge(ldB[i], 16)
        ns = 0
        for j in range(t):
            dve_ch.emit(lambda i=i, j=j: _emit2x(
                nc, pscan, out=scan_out(dslot_t[i], j),
                in0=sub_ap(a_tiles[i], j), in1=sub_ap(b_tiles[i], j)),
                wait=ns == 0)
            ns += 1
        if UNITS[i][3]:
            for j in range(t):
                dve_ch.emit(lambda i=i, j=j: _emit2x(
                    nc, pscan, out=scan_out(sbslot_t[i], j),
                    in0=sub_ap(b_tiles[i], j), in1=sub_ap(b_tiles[i], j)),
                    wait=ns == 0)
                ns += 1
        if i == n_units - 1:
            dve_ch.emit(lambda i=i: _emit2x(
                nc, pscan, out=scan_out(aslotL, 0),
                in0=sub_ap(a_tiles[i], 0), in1=sub_ap(a_tiles[i], 0)),
                wait=False)
        dve_red_marks[i] = dve_ch.n

    def emit_dve_finish(i):
        t = ts[i]
        # dot -> f32 (for the lerp scalar operand); DVE-local
        dve_ch.emit(lambda i=i, t=t: nc.vector.tensor_copy(
            dotf_t[i][:], slot_hi(dslot_t[i], t)))
        # s = sa*sb = 2|A||B| (waits once on ACT's squares+sqrts), clamp, r=1/s
        nc.vector.wait_ge(act_ch.sem, act_sq_marks[i])
        dve_ch.emit(lambda i=i: nc.vector.tensor_mul(
            sc_t[i][:], sa_t[i][:], sb_t[i][:]))
        dve_ch.emit(lambda i=i: nc.vector.tensor_scalar_max(
            sc_t[i][:], sc_t[i][:], 2 * EPS))
        dve_ch.emit(lambda i=i: nc.vector.reciprocal(r_t[i][:], sc_t[i][:]))
        # lerps write disjoint subtiles; only the first needs the chain wait
        # (RAW on r_t from the reciprocal)
        for j in range(t):
            dve_ch.emit(lambda i=i, j=j: _emit2x(
                nc, lerp2x,
                out=sub_ap(b_tiles[i], j),
                in0=sub_ap(a_tiles[i], j),
                in1=sub_ap(b_tiles[i], j),
                s0=dotf_t[i][:, j : j + 1],
                s1=r_t[i][:, j : j + 1],
                imm2=0.5,
            ), wait=j == 0)
        dve_lerp_marks[i] = dve_ch.n

    # Emission interleave: DVE runs one unit ahead on reductions; stats and
    # lerps of unit i-1 are emitted after the reductions of unit i.
    for i in range(n_units):
        emit_dve_reductions(i)
        emit_act_squares(i)
        if i > 0:
            emit_dve_finish(i - 1)
    emit_dve_finish(n_units - 1)

    # ---- Pool: SWDGE stores, deferred until ALL loads are done ----
    nc.gpsimd.wait_ge(ldB[n_units - 1], 16)
    for i, u in enumerate(UNITS):
        nc.gpsimd.wait_ge(dve_ch.sem, dve_lerp_marks[i])
        nc.gpsimd.dma_start(dram_ap(Ov, u), b_tiles[i][:]).then_inc(stS[i], 16)

    # ---- SP tail: program is done when every store has landed ----
    for i in range(n_units):
        nc.sync.wait_ge(stS[i], 16)

    nc.finalize()
    return nc


_prog_cache = {}


def _get_program():
    key = ("v4", ROWS, D)
    if key not in _prog_cache:
        _prog_cache[key] = build_program()
    return _prog_cache[key]


def make_in_maps(A, B):
    """Shard full f32 inputs row-wise into per-core bf16 in_maps."""
    A = np.asarray(A, dtype=np.float32).astype(NP_BF16)
    B = np.asarray(B, dtype=np.float32).astype(NP_BF16)
    assert A.shape == (N_FULL, D) and B.shape == (N_FULL, D)
    return [
        {
            "A": np.ascontiguousarray(A[i * ROWS : (i + 1) * ROWS]),
            "B": np.ascontiguousarray(B[i * ROWS : (i + 1) * ROWS]),
        }
        for i in range(NCORES)
    ]


def kernel(A, B):
    from concourse.bass_utils import run_bass_kernel_spmd

    nc = _get_program()
    in_maps = make_in_maps(A, B)
    res = run_bass_kernel_spmd(nc, in_maps, list(range(NCORES)))
    return np.concatenate(
        [np.asarray(res.results[i]["out"]).astype(np.float32) for i in range(NCORES)],
        axis=0,
    )
